# revision 9
# baseline (speedup 1.0000x reference)
"""Trainium2 Bass kernel for nn_Concept_ood (8 NeuronCores).

Reference computation:
    W_s = bn_tanh(concept_feature @ fc1_w + fc1_b, bn1_g, bn1_b)   # (S, E)
    W_v = bn_tanh(emb_instance @ fc2_w + fc2_b, bn2_g, bn2_b)      # (B, E)
    a_s = (W_v * fc3_w) @ W_s.T + fc3_b                            # (B, S)
    weights_v = softmax(a_s * 10, axis=1)
    emb_concept = l2norm(weights_v @ l2norm(concept_feature))      # (B, D)
    returns (emb_concept, weights_v)

Notes exploited:
  - fc1_b/fc2_b vanish inside BatchNorm (per-feature constant shifts).
  - fc3_b is a constant logit shift -> softmax invariant. Both dropped.
  - l2norm(cf) row norms are folded into the softmax weights (scale column
    s of weights by 1/(||cf_s||+eps)) so the final matmul uses raw cf.

Sharding (8 cores):
  - fc1 branch: E-sharded (each core computes 128 columns of W_s; BN1 batch
    stats are over S which is fully local per column). Needs full cf,
    transposed on-chip via PE transposes.
  - W_s^T (x fc3_w) shards AllGathered -> every core holds [E=1024, S=512].
  - fc2 branch: batch-sharded (each core owns 128 rows of emb_instance).
    BatchNorm batch stats over B=1024 need a cross-core AllReduce (sync BN)
    of per-feature sum/sumsq ([1, 2048] = 8KB).
  - a_s / softmax / final matmul / outputs: batch-sharded (128 rows/core).
All matmuls run in float32r (TF32-like PE fast path, ~2e-4 rel err).
"""

import numpy as np

import concourse.bacc as bacc
import concourse.bass as bass
import concourse.tile as tile
import concourse.mybir as mybir
import concourse.bass_utils as bass_utils

B, S, E, D = 1024, 512, 1024, 2048
P = 128
NCORES = 8
SMOOTH = 10.0
BN_EPS = 1e-5
L2_EPS = 1e-8

f32 = mybir.dt.float32
f32r = mybir.dt.float32r
AF = mybir.ActivationFunctionType
ALU = mybir.AluOpType
AX = mybir.AxisListType

KD = D // P    # 16 k-tiles over D (fc1 contraction)
KE = E // P    # 8  k-tiles over E
KS = S // P    # 4  k-tiles over S
ND = D // 512  # 4  n-chunks over D (final matmul)


def build(repeat: int = 1):
    nc = bacc.Bacc("TRN2", target_bir_lowering=False, debug=False,
                   num_devices=NCORES)

    # ---- external I/O (per-core shapes) ----
    cf = nc.dram_tensor("cf", [S, D], f32, kind="ExternalInput")
    w1 = nc.dram_tensor("w1", [D, P], f32, kind="ExternalInput")      # fc1_w[:, e_c]
    g1 = nc.dram_tensor("g1", [P, 1], f32, kind="ExternalInput")
    b1 = nc.dram_tensor("b1", [P, 1], f32, kind="ExternalInput")
    f3 = nc.dram_tensor("f3", [P, 1], f32, kind="ExternalInput")      # fc3_w[e_c]
    emb = nc.dram_tensor("emb", [P, E], f32, kind="ExternalInput")    # rows b_c
    w2 = nc.dram_tensor("w2", [E, E], f32, kind="ExternalInput")
    g2 = nc.dram_tensor("g2", [P, E // P], f32, kind="ExternalInput")
    b2 = nc.dram_tensor("b2", [P, E // P], f32, kind="ExternalInput")
    ident = nc.dram_tensor("ident", [P, P], f32, kind="ExternalInput")

    out_e = nc.dram_tensor("out_e", [P, D], f32, kind="ExternalOutput")
    out_w = nc.dram_tensor("out_w", [P, S], f32, kind="ExternalOutput")

    # ---- collective bounce buffers ----
    ag_in = nc.dram_tensor("ag_in", [P, S], f32, kind="Internal", addr_space="Local")
    ag_out = nc.dram_tensor("ag_out", [E, S], f32, kind="Internal", addr_space="Shared")
    ar_in = nc.dram_tensor("ar_in", [P, 16], f32, kind="Internal", addr_space="Local")
    ar_out = nc.dram_tensor("ar_out", [P, 16], f32, kind="Internal", addr_space="Shared")

    groups = [list(range(NCORES))]

    with tile.TileContext(nc) as tc:
        with (
            tc.tile_pool(name="const", bufs=1) as cpool,
            tc.tile_pool(name="big", bufs=1) as bpool,
            tc.tile_pool(name="work", bufs=1) as wpool,
            tc.tile_pool(name="small", bufs=1) as spool,
            tc.tile_pool(name="psA", bufs=2, space="PSUM") as psA,
            tc.tile_pool(name="psB", bufs=3, space="PSUM") as psB,
            tc.tile_pool(name="psC", bufs=2, space="PSUM") as psC,
        ):
            # ============ persistent loads ============
            id_sb = cpool.tile([P, P], f32r)
            nc.sync.dma_start(id_sb[:], ident.ap().bitcast(f32r))
            eps_sb = cpool.tile([P, 1], f32)
            nc.vector.memset(eps_sb[:], BN_EPS)

            cf_sb = bpool.tile([P, KS, D], f32r)       # cf natural (s-part tiles)
            cf_re = cf.ap().bitcast(f32r).rearrange("(t p) d -> p t d", p=P)
            for t in range(KS):
                nc.sync.dma_start(cf_sb[:, t, :], cf_re[:, t, :])

            w1_sb = bpool.tile([P, KD, P], f32r)       # fc1_w k-tiles
            nc.sync.dma_start(
                w1_sb[:], w1.ap().bitcast(f32r).rearrange("(k p) m -> p k m", p=P))

            emb_sb = bpool.tile([P, KE, P], f32r)      # emb rows (b-part)
            nc.sync.dma_start(
                emb_sb[:], emb.ap().bitcast(f32r).rearrange("p (j m) -> p j m", m=P))

            w2_sb = bpool.tile([P, KE, E], f32r)       # fc2_w k-tiles
            w2_re = w2.ap().bitcast(f32r).rearrange("(k p) n -> p k n", p=P)
            for k in range(KE):
                nc.sync.dma_start(w2_sb[:, k, :], w2_re[:, k, :])

            g1_sb = spool.tile([P, 1], f32)
            nc.sync.dma_start(g1_sb[:], g1.ap())
            b1_sb = spool.tile([P, 1], f32)
            nc.sync.dma_start(b1_sb[:], b1.ap())
            f3_sb = spool.tile([P, 1], f32)
            nc.sync.dma_start(f3_sb[:], f3.ap())
            g2_sb = spool.tile([P, KE], f32)
            nc.sync.dma_start(g2_sb[:], g2.ap())
            b2_sb = spool.tile([P, KE], f32)
            nc.sync.dma_start(b2_sb[:], b2.ap())

            for _ in range(repeat):
                body(nc, tc, bpool, wpool, spool, psA, psB, psC,
                     cf_sb, w1_sb, emb_sb, w2_sb,
                     id_sb, eps_sb, g1_sb, b1_sb, f3_sb, g2_sb, b2_sb,
                     ag_in, ag_out, ar_in, ar_out, out_e, out_w, groups)

    nc.compile()
    return nc


def body(nc, tc, bpool, wpool, spool, psA, psB, psC,
         cf_sb, w1_sb, emb_sb, w2_sb,
         id_sb, eps_sb, g1_sb, b1_sb, f3_sb, g2_sb, b2_sb,
         ag_in, ag_out, ar_in, ar_out, out_e, out_w, groups):
    # ============ cf row norms (DVE, early) ============
    cn_sb = spool.tile([P, KS], f32, tag="cn")
    nrm_scratch = wpool.tile([P, D], f32, tag="d2048")      # shared with oe
    for t in range(KS):
        nc.scalar.activation(nrm_scratch[:], cf_sb[:, t, :].bitcast(f32),
                             AF.Square, accum_out=cn_sb[:, t:t + 1])
    nc.scalar.activation(cn_sb[:], cn_sb[:], AF.Sqrt)
    nc.vector.tensor_scalar_add(cn_sb[:], cn_sb[:], L2_EPS)
    cninv_sb = spool.tile([P, KS], f32, tag="cninv")
    nc.vector.reciprocal(cninv_sb[:], cn_sb[:])

    # ============ transpose cf: cfT[p, j, s] = cf[s, 128j+p] ============
    cfT_sb = bpool.tile([P, KD, S], f32r, tag="kbuf")       # shared with wsT_all
    for j in range(KD):
        tp_ps = psA.tile([P, S], f32r, tag="psA")
        for t in range(KS):
            nc.tensor.transpose(
                tp_ps[:, t * P:(t + 1) * P], cf_sb[:, t, j * P:(j + 1) * P], id_sb[:])
        nc.vector.tensor_copy(cfT_sb[:, j, :], tp_ps[:])

    # ============ fc1: z1T[e_c, s] ============
    z1_ps = psB.tile([P, S], f32, tag="psB")
    for k in range(KD):
        nc.tensor.matmul(z1_ps[:], w1_sb[:, k, :], cfT_sb[:, k, :],
                         start=(k == 0), stop=(k == KD - 1))

    # ============ BN1 + tanh + fc3 scale (e on partitions) ============
    sum1 = spool.tile([P, 1], f32, tag="sum1")
    nc.vector.reduce_sum(sum1[:], z1_ps[:], axis=AX.X)
    ss1 = spool.tile([P, 1], f32, tag="ss1")
    bn_scratch = wpool.tile([P, 512], f32, tag="s512")
    nc.scalar.activation(bn_scratch[:], z1_ps[:], AF.Square, accum_out=ss1[:])
    mu1 = spool.tile([P, 1], f32, tag="mu1")
    nc.vector.tensor_scalar_mul(mu1[:], sum1[:], 1.0 / S)
    var1 = spool.tile([P, 1], f32, tag="var1")
    nc.vector.tensor_scalar_mul(var1[:], ss1[:], 1.0 / S)
    musq1 = spool.tile([P, 1], f32, tag="musq1")
    nc.vector.tensor_mul(musq1[:], mu1[:], mu1[:])
    nc.vector.tensor_sub(var1[:], var1[:], musq1[:])
    nc.scalar.activation(var1[:], var1[:], AF.Sqrt, bias=eps_sb[:])
    inv1 = spool.tile([P, 1], f32, tag="inv1")
    nc.vector.reciprocal(inv1[:], var1[:])
    sc1 = spool.tile([P, 1], f32, tag="sc1")
    nc.vector.tensor_mul(sc1[:], g1_sb[:], inv1[:])
    bi1 = spool.tile([P, 1], f32, tag="bi1")
    nc.vector.tensor_mul(bi1[:], mu1[:], sc1[:])
    nc.vector.tensor_sub(bi1[:], b1_sb[:], bi1[:])
    wsf_sb = wpool.tile([P, S], f32r, tag="wsf")
    nc.scalar.activation(wsf_sb[:], z1_ps[:], AF.Tanh, bias=bi1[:], scale=sc1[:])
    nc.vector.tensor_scalar_mul(wsf_sb[:], wsf_sb[:], f3_sb[:])

    # ============ AllGather W_s^T * fc3 -> [E, S] ============
    nc.sync.dma_start(ag_in.ap(), wsf_sb[:].bitcast(f32))
    nc.gpsimd.collective_compute(
        "AllGather", ALU.bypass, replica_groups=groups,
        ins=[ag_in.ap().opt()], outs=[ag_out.ap().opt()])
    wsT_all = bpool.tile([P, KE, S], f32r, tag="kbuf")      # reuses cfT slot
    nc.sync.dma_start(
        wsT_all[:], ag_out.ap().bitcast(f32r).rearrange("(k p) s -> p k s", p=P))

    # ============ transpose emb rows: embT[p, j, b] = emb[b, 128j+p] ========
    embT_sb = bpool.tile([P, KE, P], f32r, tag="embT")
    for g in range(KE // 4):
        tp2_ps = psA.tile([P, 4 * P], f32r, tag="psA")
        for jj in range(4):
            j = 4 * g + jj
            nc.tensor.transpose(
                tp2_ps[:, jj * P:(jj + 1) * P], emb_sb[:, j, :], id_sb[:])
        nc.vector.tensor_copy(embT_sb[:, 4 * g:4 * (g + 1), :], tp2_ps[:])

    # ============ fc2: z2[b_c, e] ============
    z2_sb = wpool.tile([P, E], f32r, tag="z2")
    for n in range(2):
        z2_ps = psB.tile([P, 512], f32, tag="psB")
        for k in range(KE):
            nc.tensor.matmul(z2_ps[:], embT_sb[:, k, :],
                             w2_sb[:, k, n * 512:(n + 1) * 512],
                             start=(k == 0), stop=(k == KE - 1))
        nc.vector.tensor_copy(z2_sb[:, n * 512:(n + 1) * 512], z2_ps[:])

    # ============ transpose z2 -> z2T[e, b] (pre-activation) ============
    z2T_sb = bpool.tile([P, KE, P], f32r, tag="wvT")
    for g in range(KE // 4):
        tp3_ps = psA.tile([P, 4 * P], f32r, tag="psA")
        for jj in range(4):
            j = 4 * g + jj
            nc.tensor.transpose(
                tp3_ps[:, jj * P:(jj + 1) * P], z2_sb[:, j * P:(j + 1) * P], id_sb[:])
        nc.vector.tensor_copy(z2T_sb[:, 4 * g:4 * (g + 1), :], tp3_ps[:])

    # ============ BN2 stats in e-major layout: [128, 8 sums || 8 sumsqs] ====
    stats_sb = spool.tile([P, 16], f32, tag="stats")
    st_scratch = wpool.tile([P, P], f32, tag="s512")
    for j in range(KE):
        nc.vector.reduce_sum(stats_sb[:, j:j + 1], z2T_sb[:, j, :].bitcast(f32),
                             axis=AX.X)
        nc.scalar.activation(st_scratch[:], z2T_sb[:, j, :].bitcast(f32),
                             AF.Square, accum_out=stats_sb[:, KE + j:KE + j + 1])

    # ============ AllReduce stats (sync BN) ============
    nc.sync.dma_start(ar_in.ap(), stats_sb[:])
    nc.gpsimd.collective_compute(
        "AllReduce", ALU.add, replica_groups=groups,
        ins=[ar_in.ap().opt()], outs=[ar_out.ap().opt()])
    nc.sync.dma_start(stats_sb[:], ar_out.ap())             # global stats now

    # BN2 affine params, all [128, 8] per-partition ops
    mu2 = spool.tile([P, KE], f32, tag="mu2")
    nc.vector.tensor_scalar_mul(mu2[:], stats_sb[:, 0:KE], 1.0 / B)
    var2 = spool.tile([P, KE], f32, tag="var2")
    nc.vector.tensor_scalar_mul(var2[:], stats_sb[:, KE:2 * KE], 1.0 / B)
    musq2 = spool.tile([P, KE], f32, tag="musq2")
    nc.vector.tensor_mul(musq2[:], mu2[:], mu2[:])
    nc.vector.tensor_sub(var2[:], var2[:], musq2[:])
    nc.scalar.activation(var2[:], var2[:], AF.Sqrt, bias=eps_sb[:])
    inv2 = spool.tile([P, KE], f32, tag="inv2")
    nc.vector.reciprocal(inv2[:], var2[:])
    sc2 = spool.tile([P, KE], f32, tag="sc2")
    nc.vector.tensor_mul(sc2[:], g2_sb[:], inv2[:])
    bi2 = spool.tile([P, KE], f32, tag="bi2")
    nc.vector.tensor_mul(bi2[:], mu2[:], sc2[:])
    nc.vector.tensor_sub(bi2[:], b2_sb[:], bi2[:])

    # ============ BN2 apply + tanh, fused per e-tile (in place) ============
    wvT_sb = z2T_sb
    for j in range(KE):
        nc.scalar.activation(wvT_sb[:, j, :], z2T_sb[:, j, :].bitcast(f32),
                             AF.Tanh, bias=bi2[:, j:j + 1], scale=sc2[:, j:j + 1])

    # ============ a_s rows = (W_v^T)^T @ (W_s^T * fc3) ============
    as_ps = psB.tile([P, S], f32, tag="psB")
    for k in range(KE):
        nc.tensor.matmul(as_ps[:], wvT_sb[:, k, :], wsT_all[:, k, :],
                         start=(k == 0), stop=(k == KE - 1))

    # ============ softmax (over free axis s) ============
    mx = spool.tile([P, 1], f32, tag="mx")
    nc.vector.reduce_max(mx[:], as_ps[:], axis=AX.X)
    bs = spool.tile([P, 1], f32, tag="bs")
    nc.vector.tensor_scalar_mul(bs[:], mx[:], -SMOOTH)
    wvo_sb = wpool.tile([P, S], f32r, tag="wvo")
    se = spool.tile([P, 1], f32, tag="se")
    nc.scalar.activation(wvo_sb[:], as_ps[:], AF.Exp, bias=bs[:], scale=SMOOTH,
                         accum_out=se[:])
    rec = spool.tile([P, 1], f32, tag="rec")
    nc.vector.reciprocal(rec[:], se[:])
    nc.vector.tensor_scalar_mul(wvo_sb[:], wvo_sb[:], rec[:])
    nc.sync.dma_start(out_w.ap(), wvo_sb[:].bitcast(f32))

    # ============ transpose weights rows + scale by 1/||cf_s|| ============
    wT_sb = wpool.tile([P, KS, P], f32r, tag="wT")
    tp4_ps = psA.tile([P, 4 * P], f32r, tag="psA")
    for t in range(KS):
        nc.tensor.transpose(
            tp4_ps[:, t * P:(t + 1) * P], wvo_sb[:, t * P:(t + 1) * P], id_sb[:])
    for t in range(KS):
        nc.vector.tensor_scalar_mul(
            wT_sb[:, t, :], tp4_ps[:, t * P:(t + 1) * P], cninv_sb[:, t:t + 1])

    # ============ final matmul + l2norm ============
    fs = spool.tile([P, ND], f32, tag="fs")
    oe_sb = wpool.tile([P, D], f32, tag="d2048")            # reuses nrm_scratch
    fin_scratch = wpool.tile([P, 512], f32, tag="s512")
    for n in range(ND):
        fin_ps = psB.tile([P, 512], f32, tag="psB")
        for k in range(KS):
            nc.tensor.matmul(fin_ps[:], wT_sb[:, k, :],
                             cf_sb[:, k, n * 512:(n + 1) * 512],
                             start=(k == 0), stop=(k == KS - 1))
        # evacuate unscaled (frees the PSUM slot), square-reduce on ACT
        nc.vector.tensor_copy(oe_sb[:, n * 512:(n + 1) * 512], fin_ps[:])
        nc.scalar.activation(fin_scratch[:], oe_sb[:, n * 512:(n + 1) * 512],
                             AF.Square, accum_out=fs[:, n:n + 1])
    fss = spool.tile([P, 1], f32, tag="fss")
    nc.vector.reduce_sum(fss[:], fs[:], axis=AX.X)
    nc.scalar.activation(fss[:], fss[:], AF.Sqrt)
    nc.vector.tensor_scalar_add(fss[:], fss[:], L2_EPS)
    fin_inv = spool.tile([P, 1], f32, tag="fin_inv")
    nc.vector.reciprocal(fin_inv[:], fss[:])
    for n in range(ND):
        nc.vector.tensor_scalar_mul(
            oe_sb[:, n * 512:(n + 1) * 512], oe_sb[:, n * 512:(n + 1) * 512],
            fin_inv[:])
    nc.sync.dma_start(out_e.ap(), oe_sb[:])


_BUILT = {}


def _get_nc(repeat: int = 1):
    if repeat not in _BUILT:
        _BUILT[repeat] = build(repeat)
    return _BUILT[repeat]


def _make_in_maps(inputs):
    cf = np.ascontiguousarray(np.asarray(inputs["concept_feature"], np.float32))
    fc1_w = np.asarray(inputs["fc1_w"], np.float32)
    bn1_g = np.asarray(inputs["bn1_g"], np.float32)
    bn1_b = np.asarray(inputs["bn1_b"], np.float32)
    fc3_w = np.asarray(inputs["fc3_w"], np.float32)
    emb = np.asarray(inputs["emb_instance"], np.float32)
    fc2_w = np.ascontiguousarray(np.asarray(inputs["fc2_w"], np.float32))
    bn2_g = np.ascontiguousarray(np.asarray(inputs["bn2_g"], np.float32).reshape(E // P, P).T)
    bn2_b = np.ascontiguousarray(np.asarray(inputs["bn2_b"], np.float32).reshape(E // P, P).T)
    ident = np.eye(P, dtype=np.float32)

    in_maps = []
    for c in range(NCORES):
        es = slice(P * c, P * (c + 1))
        in_maps.append({
            "cf": cf,
            "w1": np.ascontiguousarray(fc1_w[:, es]),
            "g1": np.ascontiguousarray(bn1_g[es].reshape(P, 1)),
            "b1": np.ascontiguousarray(bn1_b[es].reshape(P, 1)),
            "f3": np.ascontiguousarray(fc3_w[es].reshape(P, 1)),
            "emb": np.ascontiguousarray(emb[es]),
            "w2": fc2_w,
            "g2": bn2_g,
            "b2": bn2_b,
            "ident": ident,
        })
    return in_maps


def kernel(**inputs):
    nc = _get_nc(1)
    in_maps = _make_in_maps(inputs)
    res = bass_utils.run_bass_kernel_spmd(nc, in_maps, core_ids=list(range(NCORES)))
    emb_concept = np.concatenate([res.results[c]["out_e"] for c in range(NCORES)], axis=0)
    weights_v = np.concatenate([res.results[c]["out_w"] for c in range(NCORES)], axis=0)
    return (emb_concept, weights_v)
